# revision 73
# baseline (speedup 1.0000x reference)
"""Causal self-attention (B=2, T=4096, C=512, H=8, Dh=64) on 8 trn2 cores.

Sharding: core = (batch, head-pair). 2 batches x 4 head-pairs = 8 cores.
Each core computes q/k/v projections for its 2 heads, causal attention in
S^T ([k, q]) layout, and a row-parallel slice of the output projection.
Host sums the 4 bf16 partial outputs per batch (+ b_out) and stacks.

bf16 pipeline (PSUM accumulation stays f32 where it matters), measured
~183 us/core on trn2 (from a 364 us starting point; ACT exp floor ~139us,
PE ~153us busy at 93% density):
  - x / weights arrive bf16; Q/K/V produced bf16 (projection matmuls
    accumulate f32 in PSUM, DVE bias-add casts to bf16).
  - S^T chunk psum is per-CHUNK [128, 2(heads), 512]: the two matmuls
    filling it sit on different PE row groups (lhsT base partition 0 /
    64, K=64 each) and execute CONCURRENTLY in the array (~2x S
    throughput); the scheduler keeps them adjacent because each exp is
    unblocked by one tile.
  - exp on ACT per chunk at [128, <=1024] (PSUM source, scale fused);
    diagonal chunks narrow S/exp/Y to skip the fully-masked 128r query
    prefix, and only the [128,128] triangle block is mask-multiplied
    (DVE bf16).
  - YT[h][65, 512] += V_chunk @ expS in f32 PSUM (row 64 = softmax
    denominator via an appended ones column in V).
  - Deferred normalization: yt_ps is evicted immediately (bf16 YTu +
    den row), the denominator is partition-broadcast on the otherwise
    idle GPSIMD, and the ~51-ULP reciprocal_approx_fast (5x faster than
    the iterative DVE divide) + scale run one tile behind, so neither
    PE nor the DVE queue head ever waits on the reciprocal. The last
    tile instead uses a PE K=1 broadcast matmul + PSUM-direct ops for
    the shortest serial tail.
  - Out-projection: both heads stacked on partitions (YTn [128, 512],
    woT [128, 512]) -> ONE K=128 matmul per 128-query block.
  - 1-deep S software pipeline: each pair body first emits the NEXT
    pair's S quad + exp (crossing tile boundaries), so ACT always has
    its next input in flight and PE/ACT never idle against each other.
"""

import os
import sys

import numpy as np

for _p in ("/opt/trn_rl_repo",):
    if os.path.isdir(_p) and _p not in sys.path:
        sys.path.insert(0, _p)

os.environ.setdefault("MYCRO_LOCAL_CACHE", "1")


def _ensure_ntff_hook():
    """bass_utils' trace path imports antenv.axon_hooks; some images lack
    it. Recreate the module with the same ctypes hook if missing."""
    try:
        import antenv.axon_hooks  # noqa: F401

        return
    except ImportError:
        pass
    try:
        import types

        import antenv  # noqa: F401
        from trn_agent_boot.trn_boot import _ntff_profile_via_ctypes

        hook = _ntff_profile_via_ctypes("/opt/axon/libaxon_pjrt.so")
        mod = types.ModuleType("antenv.axon_hooks")
        mod.get_axon_ntff_profile_hook = lambda: hook
        mod.set_axon_ntff_profile_hook = lambda h: None
        sys.modules["antenv.axon_hooks"] = mod
    except Exception:
        pass


_ensure_ntff_hook()

import concourse.bass as bass  # noqa: E402
from concourse import bacc  # noqa: E402
import concourse.mybir as mybir  # noqa: E402
import concourse.tile as tile  # noqa: E402
from concourse.bass_utils import run_bass_kernel_spmd  # noqa: E402

F32 = mybir.dt.float32
F32R = mybir.dt.float32r
BF16 = mybir.dt.bfloat16

B, T, C, H, DH = 2, 4096, 512, 8, 64
HEADS_PER_CORE = 2
HD = HEADS_PER_CORE * DH  # 128: head dims owned by one core
N_CORES = 8
QT_TILE = 512  # queries per attention tile
KC = 128  # keys per chunk (contraction granularity)
N_QT = T // QT_TILE  # 8
N_KC = T // KC  # 32
CK = C // 128  # 4 contraction chunks for the projections
SCALE = 1.0 / float(np.sqrt(DH))


def build_program():
    nc = bacc.Bacc(None)

    xT = nc.declare_dram_parameter("xT", [C, T], BF16, isOutput=False)
    wqT = nc.declare_dram_parameter("wqT", [C, HD], BF16, isOutput=False)
    wkT = nc.declare_dram_parameter("wkT", [C, HD], BF16, isOutput=False)
    wvT = nc.declare_dram_parameter("wvT", [C, HD], BF16, isOutput=False)
    # woT[p, j]: rows of w_out for this core's head dims; rows 0-63 = head0
    # dims, 64-127 = head1 dims (matches the stacked YTn layout, so the
    # out-projection is ONE K=128 matmul summing both heads).
    woT = nc.declare_dram_parameter("woT", [HD, C], BF16, isOutput=False)
    bq = nc.declare_dram_parameter("bq", [HD], F32, isOutput=False)
    bk = nc.declare_dram_parameter("bk", [HD], F32, isOutput=False)
    bv = nc.declare_dram_parameter("bv", [HD], F32, isOutput=False)
    out = nc.declare_dram_parameter("out", [T, C], BF16, isOutput=True)

    with tile.TileContext(nc) as tc:
        with (
            tc.tile_pool(name="singles", bufs=1) as singles,
            tc.tile_pool(name="xin", bufs=8) as xin,
            tc.tile_pool(name="exps", bufs=6) as exps,
            tc.tile_pool(name="osb", bufs=3) as osb,
            tc.tile_pool(name="norm", bufs=2) as norm,
            tc.tile_pool(name="ps_proj", bufs=2, space="PSUM") as ps_proj,
            tc.tile_pool(name="ps_s", bufs=2, space="PSUM") as ps_s,
            tc.tile_pool(name="ps_yt", bufs=1, space="PSUM") as ps_yt,
        ):
            # ---- resident inputs (x0 + q/k weights first: they gate the
            # first PE work) --------------------------------------------
            xT_ap = xT.rearrange("(ko p) t -> p ko t", p=128)
            xt_first = xin.tile([128, CK, QT_TILE], BF16, tag="xt", name="xt_first")
            nc.sync.dma_start(xt_first, xT_ap[:, :, bass.ts(0, QT_TILE)])
            wqT_sb = singles.tile([128, CK, HD], BF16)
            nc.sync.dma_start(wqT_sb, wqT.rearrange("(ko p) m -> p ko m", p=128))
            wkT_sb = singles.tile([128, CK, HD], BF16)
            nc.sync.dma_start(wkT_sb, wkT.rearrange("(ko p) m -> p ko m", p=128))
            wvT_sb = singles.tile([128, CK, HD], BF16)
            nc.sync.dma_start(wvT_sb, wvT.rearrange("(ko p) m -> p ko m", p=128))
            woT_sb = singles.tile([HD, C], BF16)
            nc.sync.dma_start(woT_sb, woT[:])

            bq_col = singles.tile([128, 1], F32)
            nc.sync.dma_start(bq_col, bq.rearrange("(p one) -> p one", one=1))
            bk_col = singles.tile([128, 1], F32)
            nc.sync.dma_start(bk_col, bk.rearrange("(p one) -> p one", one=1))
            bv_row = singles.tile([1, HD], F32)
            nc.sync.dma_start(bv_row, bv[None, :])

            ones_f32 = singles.tile([128, 128], F32)
            nc.vector.memset(ones_f32, 1.0)


            ones_bf = singles.tile([128, 4], BF16)
            nc.vector.tensor_copy(ones_bf, ones_f32[:, 0:4])

            # [128,128] causal triangle: tri[k, qq] = 1 if k <= qq.
            # Built in F32 (affine_select needs it), then cast.
            tri_f32 = singles.tile([128, 128], F32)
            nc.vector.memset(tri_f32, 1.0)
            nc.gpsimd.affine_select(
                out=tri_f32,
                in_=tri_f32,
                compare_op=mybir.AluOpType.is_ge,
                fill=0.0,
                base=0,
                pattern=[[1, 128]],
                channel_multiplier=-1,
            )
            tri_bf = singles.tile([128, 128], BF16)
            nc.vector.tensor_copy(tri_bf, tri_f32)

            # broadcast bv across partitions on gpsimd
            bias_v_sb = singles.tile([128, HD], F32)
            nc.gpsimd.partition_broadcast(bias_v_sb, bv_row)
            bias_v2 = bias_v_sb.rearrange("p (h x) -> p h x", h=2)

            # per-tile storage (separate tile objects -> precise deps)
            QT_t = [
                singles.tile([128, QT_TILE], BF16, name=f"qtt{i}", tag=f"qtt{i}")
                for i in range(N_QT)
            ]
            KT_t = [
                singles.tile([128, QT_TILE], BF16, name=f"ktt{i}", tag=f"ktt{i}")
                for i in range(N_QT)
            ]
            # V chunks in [k, d] layout; per tile: 4 chunks of
            # [V0 | ones | V1 | ones] (65-column stride per head slice)
            V_t = [
                singles.tile([128, 4, 130], BF16, name=f"vt{i}", tag=f"vt{i}")
                for i in range(N_QT)
            ]
            # unnormalized attention outputs + denominators (deferred norm)
            YTu_t = [
                [
                    singles.tile(
                        [64, QT_TILE], BF16, name=f"ytu{h}_{i}", tag=f"ytu{h}_{i}"
                    )
                    for i in range(N_QT)
                ]
                for h in range(2)
            ]
            den_t = [
                [
                    singles.tile([1, QT_TILE], F32, name=f"den{h}_{i}", tag=f"den{h}_{i}")
                    for i in range(N_QT)
                ]
                for h in range(2)
            ]
            # normalized YT, both heads stacked on partitions (h0: 0-63,
            # h1: 64-127) so the out-projection contracts K=128 in one shot
            YTn_t = [
                singles.tile([128, QT_TILE], BF16, name=f"ytn{i}", tag=f"ytn{i}")
                for i in range(N_QT)
            ]
            for i in range(N_QT):
                nc.vector.tensor_copy(V_t[i][:, :, 64:65], ones_bf[:, :, None])
                nc.vector.tensor_copy(V_t[i][:, :, 129:130], ones_bf[:, :, None])

            def emit_qproj(qt, xt):
                ps_q = ps_proj.tile([128, QT_TILE], F32, tag="psproj", name="ps_q")
                for kc in range(CK):
                    nc.tensor.matmul(
                        ps_q,
                        wqT_sb[:, kc, :],
                        xt[:, kc, :],
                        start=(kc == 0),
                        stop=(kc == CK - 1),
                    )
                nc.vector.tensor_scalar_add(QT_t[qt][:], ps_q, bq_col)

            def emit_kproj(qt, xt):
                ps_k = ps_proj.tile([128, QT_TILE], F32, tag="psproj", name="ps_k")
                for kc in range(CK):
                    nc.tensor.matmul(
                        ps_k,
                        wkT_sb[:, kc, :],
                        xt[:, kc, :],
                        start=(kc == 0),
                        stop=(kc == CK - 1),
                    )
                nc.vector.tensor_scalar_add(KT_t[qt][:], ps_k, bk_col)

            def emit_vproj(qt, xt, sv):
                ps_v = ps_proj.tile([128, HD], F32, tag="psproj", name="ps_v")
                for kc in range(CK):
                    nc.tensor.matmul(
                        ps_v,
                        xt[:, kc, bass.ts(sv, 128)],
                        wvT_sb[:, kc, :],
                        start=(kc == 0),
                        stop=(kc == CK - 1),
                    )
                vt = V_t[qt]
                v_vals = bass.AP(
                    tensor=vt.tensor,
                    offset=vt.offset,
                    ap=[vt.ap[0], vt.ap[1], [65, 2], [1, 64]],
                )
                nc.vector.tensor_add(
                    v_vals[:, sv],
                    ps_v.rearrange("p (h x) -> p h x", h=2),
                    bias_v2,
                )

            def emit_norm_a(qt, yt_ps):
                # evict yt_ps fast (den row + unnormalized YT), then
                # broadcast den across partitions on the idle gpsimd so
                # neither PE nor the DVE queue head ever waits on it.
                # per-head order: head0's psum bank frees before head1's
                # copies start, unblocking the next tile's first Y matmul
                for h in range(2):
                    nc.vector.tensor_copy(den_t[h][qt][:], yt_ps[h][64:65, :])
                    nc.vector.tensor_copy(YTu_t[h][qt], yt_ps[h][0:64, :])
                den_bc = []
                for h in range(2):
                    bc = norm.tile(
                        [64, QT_TILE], F32, tag=f"denbc{h}", name=f"denbc{h}"
                    )
                    nc.gpsimd.partition_broadcast(bc, den_t[h][qt][:])
                    den_bc.append(bc)
                return den_bc

            def emit_norm_b(qt, den_bc):
                # ~51-ULP reciprocal (5x faster than the iterative divide),
                # then scale the unnormalized attention rows.
                for h in range(2):
                    rec_sb = norm.tile(
                        [64, QT_TILE], F32, tag=f"rec{h}", name=f"rec{h}"
                    )
                    nc.vector.reciprocal_approx_fast(rec_sb, den_bc[h])
                    nc.vector.tensor_mul(
                        YTn_t[qt][64 * h : 64 * h + 64, :],
                        YTu_t[h][qt][:],
                        rec_sb,
                    )

            def emit_norm_tail(qt, yt_ps):
                # last tile: PE is idle, so broadcast the denominator with a
                # K=1 matmul and normalize straight out of PSUM -- shortest
                # serial chain before the final out-projection.
                ones_r = norm.tile([1, 64], F32R, tag="onesr", name="onesr")
                with nc.allow_low_precision(reason="f32r ones for tail bcast"):
                    nc.vector.tensor_copy(ones_r, ones_f32[0:1, 0:64])
                for h in range(2):
                    den_r = norm.tile(
                        [1, QT_TILE], F32R, tag=f"denr{h}", name=f"denr{h}"
                    )
                    with nc.allow_low_precision(
                        reason="tf32-rounded softmax denominator for the "
                        "tail broadcast matmul"
                    ):
                        nc.vector.tensor_copy(den_r, yt_ps[h][64:65, :])
                    bc_ps = ps_proj.tile(
                        [64, QT_TILE], F32, tag="psproj", name="bc_tail"
                    )
                    nc.tensor.matmul(
                        bc_ps,
                        ones_r,
                        den_r,
                        start=True,
                        stop=True,
                    )
                    rec_sb = norm.tile(
                        [64, QT_TILE], F32, tag=f"rec{h}", name=f"rect{h}"
                    )
                    nc.vector.reciprocal_approx_fast(rec_sb, bc_ps)
                    nc.vector.tensor_mul(
                        YTn_t[qt][64 * h : 64 * h + 64, :],
                        yt_ps[h][0:64, :],
                        rec_sb,
                    )

            def emit_outproj_sv(qt, sv):
                tc8 = qt * (QT_TILE // 128) + sv
                ps_o = ps_proj.tile([128, C], F32, tag="psproj", name="ps_o")
                nc.tensor.matmul(
                    ps_o,
                    YTn_t[qt][:, bass.ts(sv, 128)],
                    woT_sb,
                    start=True,
                    stop=True,
                )
                o_sb = osb.tile([128, C], BF16, tag="osb")
                nc.vector.tensor_copy(o_sb, ps_o)
                nc.sync.dma_start(out[bass.ts(tc8, 128), :], o_sb)

            xt_tiles = {0: xt_first}

            def emit_xt(i):
                if i not in xt_tiles and i < N_QT:
                    xt_i = xin.tile(
                        [128, CK, QT_TILE], BF16, tag="xt", name=f"xt{i}"
                    )
                    nc.sync.dma_start(xt_i, xT_ap[:, :, bass.ts(i, QT_TILE)])
                    xt_tiles[i] = xt_i

            def emit_s_exp(qt2, pair):
                """S^T quad + exp + causal mask for (query tile qt2, chunk
                pair). The S psum tile is per-CHUNK (both heads): the two
                matmuls filling it hit different PE row groups (lhsT base
                partitions 0 / 64) and run concurrently in the array, and
                the scheduler keeps them adjacent because each exp is
                unblocked by one tile. Diagonal chunks skip the fully-
                masked 128r query prefix (excluded from Y, never read) and
                mask only the [128,128] triangle block."""
                e_list = []
                for sub in range(2):
                    c = pair * 2 + sub
                    r = c - 4 * qt2
                    off = KC * r if r > 0 else 0
                    s_ps = ps_s.tile(
                        [128, 2, QT_TILE], F32, tag="s", name=f"s{sub}"
                    )
                    for h in range(2):
                        hp = slice(h * 64, h * 64 + 64)
                        nc.tensor.matmul(
                            s_ps[:, h, off:],
                            KT_t[c // 4][hp, bass.ts(c % 4, KC)],
                            QT_t[qt2][hp, off:],
                            start=True,
                            stop=True,
                        )
                    e_sb = exps.tile(
                        [128, 2, QT_TILE], BF16, tag="e", name=f"e{sub}"
                    )
                    nc.scalar.activation(
                        e_sb[:, :, off:],
                        s_ps[:, :, off:],
                        mybir.ActivationFunctionType.Exp,
                        scale=SCALE,
                    )
                    if r >= 0:
                        for h in range(2):
                            nc.vector.tensor_mul(
                                e_sb[:, h, bass.ts(r, KC)],
                                e_sb[:, h, bass.ts(r, KC)],
                                tri_bf,
                            )
                    e_list.append(e_sb)
                return e_list

            den_bcs = {}
            pending_e = {}
            for i in range(1, N_QT):
                emit_xt(i)
            # processing order: small tile 1 LAST as the pipeline drain
            # (tile 0 still first -- unchanged ramp-up); key-tile 1's K/V
            # are projected during tile 0 instead.
            ORDER = [0, 2, 3, 4, 5, 6, 7, 1]
            for idx, qt in enumerate(ORDER):
                xt = xt_tiles[qt]
                prev_t = ORDER[idx - 1] if idx > 0 else None
                next_t = ORDER[idx + 1] if idx + 1 < N_QT else None
                if idx == 0:
                    emit_qproj(qt, xt)
                    emit_kproj(qt, xt)
                    pending_e[(0, 0)] = emit_s_exp(0, 0)
                    for sv in range(4):
                        emit_vproj(qt, xt, sv)
                    emit_kproj(1, xt_tiles[1])

                yt_ps = [
                    ps_yt.tile([128, QT_TILE], F32, tag=f"yt{h}", name=f"yt{h}")
                    for h in range(2)
                ]
                n_pairs = 2 * (qt + 1)
                outproj_at = {}
                for sv in range(4):
                    outproj_at.setdefault(min(3 + sv, n_pairs - 1), []).append(sv)
                for pair in range(n_pairs):
                    e_sb = pending_e.pop((qt, pair), None)
                    if e_sb is None:
                        e_sb = emit_s_exp(qt, pair)
                    if pair == min(2, n_pairs - 1) and next_t is not None:
                        emit_qproj(next_t, xt_tiles[next_t])
                    # 1-deep S pipeline: emit the NEXT pair's S quad + exp
                    # before this pair's projections and Y matmuls, so ACT
                    # always has the next exp input ready.
                    if pair + 1 < n_pairs:
                        pending_e[(qt, pair + 1)] = emit_s_exp(qt, pair + 1)
                    elif next_t is not None:
                        pending_e[(next_t, 0)] = emit_s_exp(next_t, 0)
                    # pipelined projections / out-proj for other tiles
                    if idx == 0:
                        # key-tile 1's V during tile 0's two pairs
                        emit_vproj(1, xt_tiles[1], 2 * pair)
                        emit_vproj(1, xt_tiles[1], 2 * pair + 1)
                    if pair == 0 and idx > 0 and qt != 1:
                        emit_kproj(qt, xt)
                    if idx > 0 and qt != 1 and pair < 4:
                        emit_vproj(qt, xt, pair)
                    for h in range(2):
                        for sub in range(2):
                            c = pair * 2 + sub
                            r = c - 4 * qt
                            off = KC * r if r > 0 else 0
                            nc.tensor.matmul(
                                yt_ps[h][0:65, off:],
                                V_t[c // 4][:, c % 4, h * 65 : h * 65 + 65],
                                e_sb[sub][:, h, off:],
                                start=(pair == 0 and sub == 0),
                                stop=(pair == n_pairs - 1 and sub == 1),
                            )
                    if pair == 1 and prev_t is not None:
                        emit_norm_b(prev_t, den_bcs[prev_t])
                    if prev_t is not None:
                        for sv in outproj_at.get(pair, []):
                            emit_outproj_sv(prev_t, sv)

                # ---- evict yt_ps fast + deferred normalization ----
                if next_t is not None:
                    den_bcs[qt] = emit_norm_a(qt, yt_ps)
                else:
                    emit_norm_tail(qt, yt_ps)
            for sv in range(4):
                emit_outproj_sv(ORDER[-1], sv)

    return nc


_PROGRAM = None


def _get_program():
    global _PROGRAM
    if _PROGRAM is None:
        _PROGRAM = build_program()
        if not _PROGRAM.is_finalized():
            _PROGRAM.finalize()
    return _PROGRAM


def make_in_maps(x, w_qkv, b_qkv, w_out, b_out):
    """Shard the full inputs into per-core input maps."""
    import ml_dtypes

    bf16 = ml_dtypes.bfloat16
    x = np.ascontiguousarray(x, dtype=np.float32)
    w_qkv = np.ascontiguousarray(w_qkv, dtype=np.float32)
    b_qkv = np.ascontiguousarray(b_qkv, dtype=np.float32)
    w_out = np.ascontiguousarray(w_out, dtype=np.float32)

    wq = w_qkv[0:C]  # [C, C] rows = q features
    wk = w_qkv[C : 2 * C]
    wv = w_qkv[2 * C : 3 * C]
    bq_full = b_qkv[0:C]
    bk_full = b_qkv[C : 2 * C]
    bv_full = b_qkv[2 * C : 3 * C]

    xT_b = [np.ascontiguousarray(x[b].T.astype(bf16)) for b in range(B)]

    in_maps = []
    for core in range(N_CORES):
        b = core // 4
        g = core % 4
        rows = slice(g * HD, (g + 1) * HD)  # this core's head dims
        woT = np.ascontiguousarray(w_out[:, rows].T.astype(bf16))  # [HD, C]
        in_maps.append(
            {
                "xT": xT_b[b],
                "wqT": np.ascontiguousarray(wq[rows].T.astype(bf16)),
                "wkT": np.ascontiguousarray(wk[rows].T.astype(bf16)),
                "wvT": np.ascontiguousarray(wv[rows].T.astype(bf16)),
                "woT": woT,
                "bq": np.ascontiguousarray(bq_full[rows]),
                "bk": np.ascontiguousarray(bk_full[rows]),
                "bv": np.ascontiguousarray(bv_full[rows]),
            }
        )
    return in_maps


def kernel(x, w_qkv, b_qkv, w_out, b_out, _trace=False, _trace_kwargs=None):
    in_maps = make_in_maps(x, w_qkv, b_qkv, w_out, b_out)
    nc = _get_program()
    res = run_bass_kernel_spmd(
        nc,
        in_maps,
        list(range(N_CORES)),
        trace=_trace,
        **(_trace_kwargs or {}),
    )
    outs = [res.results[c]["out"].astype(np.float32) for c in range(N_CORES)]
    bo = np.asarray(b_out, dtype=np.float32)
    # unshard: sum the 4 row-parallel partials per batch (+ bias), stack
    y = np.stack(
        [
            outs[0] + outs[1] + outs[2] + outs[3] + bo,
            outs[4] + outs[5] + outs[6] + outs[7] + bo,
        ]
    ).astype(np.float32)
    if _trace:
        return y, res
    return y


# revision 74
# speedup vs baseline: 1.0013x; 1.0013x over previous
"""Causal self-attention (B=2, T=4096, C=512, H=8, Dh=64) on 8 trn2 cores.

Sharding: core = (batch, head-pair). 2 batches x 4 head-pairs = 8 cores.
Each core computes q/k/v projections for its 2 heads, causal attention in
S^T ([k, q]) layout, and a row-parallel slice of the output projection.
Host sums the 4 bf16 partial outputs per batch (+ b_out) and stacks.

bf16 pipeline (PSUM accumulation stays f32 where it matters), measured
~183 us/core on trn2 (from a 364 us starting point; ACT exp floor ~139us,
PE ~153us busy at 93% density):
  - x / weights arrive bf16; Q/K/V produced bf16 (projection matmuls
    accumulate f32 in PSUM, DVE bias-add casts to bf16).
  - S^T chunk psum is per-CHUNK [128, 2(heads), 512]: the two matmuls
    filling it sit on different PE row groups (lhsT base partition 0 /
    64, K=64 each) and execute CONCURRENTLY in the array (~2x S
    throughput); the scheduler keeps them adjacent because each exp is
    unblocked by one tile.
  - exp on ACT per chunk at [128, <=1024] (PSUM source, scale fused);
    diagonal chunks narrow S/exp/Y to skip the fully-masked 128r query
    prefix, and only the [128,128] triangle block is mask-multiplied
    (DVE bf16).
  - YT[h][65, 512] += V_chunk @ expS in f32 PSUM (row 64 = softmax
    denominator via an appended ones column in V).
  - Deferred normalization: yt_ps is evicted immediately (bf16 YTu +
    den row), the denominator is partition-broadcast on the otherwise
    idle GPSIMD, and the ~51-ULP reciprocal_approx_fast (5x faster than
    the iterative DVE divide) + scale run one tile behind, so neither
    PE nor the DVE queue head ever waits on the reciprocal. The last
    tile instead uses a PE K=1 broadcast matmul + PSUM-direct ops for
    the shortest serial tail.
  - Out-projection: both heads stacked on partitions (YTn [128, 512],
    woT [128, 512]) -> ONE K=128 matmul per 128-query block.
  - 1-deep S software pipeline: each pair body first emits the NEXT
    pair's S quad + exp (crossing tile boundaries), so ACT always has
    its next input in flight and PE/ACT never idle against each other.
"""

import os
import sys

import numpy as np

for _p in ("/opt/trn_rl_repo",):
    if os.path.isdir(_p) and _p not in sys.path:
        sys.path.insert(0, _p)

os.environ.setdefault("MYCRO_LOCAL_CACHE", "1")


def _ensure_ntff_hook():
    """bass_utils' trace path imports antenv.axon_hooks; some images lack
    it. Recreate the module with the same ctypes hook if missing."""
    try:
        import antenv.axon_hooks  # noqa: F401

        return
    except ImportError:
        pass
    try:
        import types

        import antenv  # noqa: F401
        from trn_agent_boot.trn_boot import _ntff_profile_via_ctypes

        hook = _ntff_profile_via_ctypes("/opt/axon/libaxon_pjrt.so")
        mod = types.ModuleType("antenv.axon_hooks")
        mod.get_axon_ntff_profile_hook = lambda: hook
        mod.set_axon_ntff_profile_hook = lambda h: None
        sys.modules["antenv.axon_hooks"] = mod
    except Exception:
        pass


_ensure_ntff_hook()

import concourse.bass as bass  # noqa: E402
from concourse import bacc  # noqa: E402
import concourse.mybir as mybir  # noqa: E402
import concourse.tile as tile  # noqa: E402
from concourse.bass_utils import run_bass_kernel_spmd  # noqa: E402

F32 = mybir.dt.float32
F32R = mybir.dt.float32r
BF16 = mybir.dt.bfloat16

B, T, C, H, DH = 2, 4096, 512, 8, 64
HEADS_PER_CORE = 2
HD = HEADS_PER_CORE * DH  # 128: head dims owned by one core
N_CORES = 8
QT_TILE = 512  # queries per attention tile
KC = 128  # keys per chunk (contraction granularity)
N_QT = T // QT_TILE  # 8
N_KC = T // KC  # 32
CK = C // 128  # 4 contraction chunks for the projections
SCALE = 1.0 / float(np.sqrt(DH))


def build_program():
    nc = bacc.Bacc(None)

    xT = nc.declare_dram_parameter("xT", [C, T], BF16, isOutput=False)
    wqT = nc.declare_dram_parameter("wqT", [C, HD], BF16, isOutput=False)
    wkT = nc.declare_dram_parameter("wkT", [C, HD], BF16, isOutput=False)
    wvT = nc.declare_dram_parameter("wvT", [C, HD], BF16, isOutput=False)
    # woT[p, j]: rows of w_out for this core's head dims; rows 0-63 = head0
    # dims, 64-127 = head1 dims (matches the stacked YTn layout, so the
    # out-projection is ONE K=128 matmul summing both heads).
    woT = nc.declare_dram_parameter("woT", [HD, C], BF16, isOutput=False)
    bq = nc.declare_dram_parameter("bq", [HD], F32, isOutput=False)
    bk = nc.declare_dram_parameter("bk", [HD], F32, isOutput=False)
    bv = nc.declare_dram_parameter("bv", [HD], F32, isOutput=False)
    out = nc.declare_dram_parameter("out", [T, C], BF16, isOutput=True)

    with tile.TileContext(nc) as tc:
        with (
            tc.tile_pool(name="singles", bufs=1) as singles,
            tc.tile_pool(name="xin", bufs=8) as xin,
            tc.tile_pool(name="exps", bufs=6) as exps,
            tc.tile_pool(name="osb", bufs=3) as osb,
            tc.tile_pool(name="norm", bufs=2) as norm,
            tc.tile_pool(name="ps_proj", bufs=2, space="PSUM") as ps_proj,
            tc.tile_pool(name="ps_s", bufs=2, space="PSUM") as ps_s,
            tc.tile_pool(name="ps_yt", bufs=1, space="PSUM") as ps_yt,
        ):
            # ---- resident inputs (x0 + q/k weights first: they gate the
            # first PE work) --------------------------------------------
            xT_ap = xT.rearrange("(ko p) t -> p ko t", p=128)
            xt_first = xin.tile([128, CK, QT_TILE], BF16, tag="xt", name="xt_first")
            nc.sync.dma_start(xt_first, xT_ap[:, :, bass.ts(0, QT_TILE)])
            wqT_sb = singles.tile([128, CK, HD], BF16)
            nc.sync.dma_start(wqT_sb, wqT.rearrange("(ko p) m -> p ko m", p=128))
            wkT_sb = singles.tile([128, CK, HD], BF16)
            nc.sync.dma_start(wkT_sb, wkT.rearrange("(ko p) m -> p ko m", p=128))
            wvT_sb = singles.tile([128, CK, HD], BF16)
            nc.sync.dma_start(wvT_sb, wvT.rearrange("(ko p) m -> p ko m", p=128))
            woT_sb = singles.tile([HD, C], BF16)
            nc.sync.dma_start(woT_sb, woT[:])

            bq_col = singles.tile([128, 1], F32)
            nc.sync.dma_start(bq_col, bq.rearrange("(p one) -> p one", one=1))
            bk_col = singles.tile([128, 1], F32)
            nc.sync.dma_start(bk_col, bk.rearrange("(p one) -> p one", one=1))
            bv_row = singles.tile([1, HD], F32)
            nc.sync.dma_start(bv_row, bv[None, :])

            ones_f32 = singles.tile([128, 128], F32)
            nc.vector.memset(ones_f32, 1.0)


            ones_bf = singles.tile([128, 4], BF16)
            nc.vector.tensor_copy(ones_bf, ones_f32[:, 0:4])

            # [128,128] causal triangle: tri[k, qq] = 1 if k <= qq.
            # Built in F32 (affine_select needs it), then cast.
            tri_f32 = singles.tile([128, 128], F32)
            nc.vector.memset(tri_f32, 1.0)
            nc.gpsimd.affine_select(
                out=tri_f32,
                in_=tri_f32,
                compare_op=mybir.AluOpType.is_ge,
                fill=0.0,
                base=0,
                pattern=[[1, 128]],
                channel_multiplier=-1,
            )
            tri_bf = singles.tile([128, 128], BF16)
            nc.vector.tensor_copy(tri_bf, tri_f32)

            # broadcast bv across partitions on gpsimd
            bias_v_sb = singles.tile([128, HD], F32)
            nc.gpsimd.partition_broadcast(bias_v_sb, bv_row)
            bias_v2 = bias_v_sb.rearrange("p (h x) -> p h x", h=2)

            # per-tile storage (separate tile objects -> precise deps)
            QT_t = [
                singles.tile([128, QT_TILE], BF16, name=f"qtt{i}", tag=f"qtt{i}")
                for i in range(N_QT)
            ]
            KT_t = [
                singles.tile([128, QT_TILE], BF16, name=f"ktt{i}", tag=f"ktt{i}")
                for i in range(N_QT)
            ]
            # V chunks in [k, d] layout; per tile: 4 chunks of
            # [V0 | ones | V1 | ones] (65-column stride per head slice)
            V_t = [
                singles.tile([128, 4, 130], BF16, name=f"vt{i}", tag=f"vt{i}")
                for i in range(N_QT)
            ]
            # unnormalized attention outputs + denominators (deferred norm)
            YTu_t = [
                [
                    singles.tile(
                        [64, QT_TILE], BF16, name=f"ytu{h}_{i}", tag=f"ytu{h}_{i}"
                    )
                    for i in range(N_QT)
                ]
                for h in range(2)
            ]
            den_t = [
                [
                    singles.tile([1, QT_TILE], F32, name=f"den{h}_{i}", tag=f"den{h}_{i}")
                    for i in range(N_QT)
                ]
                for h in range(2)
            ]
            # normalized YT, both heads stacked on partitions (h0: 0-63,
            # h1: 64-127) so the out-projection contracts K=128 in one shot
            YTn_t = [
                singles.tile([128, QT_TILE], BF16, name=f"ytn{i}", tag=f"ytn{i}")
                for i in range(N_QT)
            ]
            for i in range(N_QT):
                nc.vector.tensor_copy(V_t[i][:, :, 64:65], ones_bf[:, :, None])
                nc.vector.tensor_copy(V_t[i][:, :, 129:130], ones_bf[:, :, None])

            def emit_qproj(qt, xt):
                ps_q = ps_proj.tile([128, QT_TILE], F32, tag="psproj", name="ps_q")
                for kc in range(CK):
                    nc.tensor.matmul(
                        ps_q,
                        wqT_sb[:, kc, :],
                        xt[:, kc, :],
                        start=(kc == 0),
                        stop=(kc == CK - 1),
                    )
                nc.vector.tensor_scalar_add(QT_t[qt][:], ps_q, bq_col)

            def emit_kproj(qt, xt):
                ps_k = ps_proj.tile([128, QT_TILE], F32, tag="psproj", name="ps_k")
                for kc in range(CK):
                    nc.tensor.matmul(
                        ps_k,
                        wkT_sb[:, kc, :],
                        xt[:, kc, :],
                        start=(kc == 0),
                        stop=(kc == CK - 1),
                    )
                nc.vector.tensor_scalar_add(KT_t[qt][:], ps_k, bk_col)

            def emit_vproj(qt, xt, sv):
                ps_v = ps_proj.tile([128, HD], F32, tag="psproj", name="ps_v")
                for kc in range(CK):
                    nc.tensor.matmul(
                        ps_v,
                        xt[:, kc, bass.ts(sv, 128)],
                        wvT_sb[:, kc, :],
                        start=(kc == 0),
                        stop=(kc == CK - 1),
                    )
                vt = V_t[qt]
                v_vals = bass.AP(
                    tensor=vt.tensor,
                    offset=vt.offset,
                    ap=[vt.ap[0], vt.ap[1], [65, 2], [1, 64]],
                )
                nc.vector.tensor_add(
                    v_vals[:, sv],
                    ps_v.rearrange("p (h x) -> p h x", h=2),
                    bias_v2,
                )

            def emit_norm_a(qt, yt_ps):
                # evict yt_ps fast (den row + unnormalized YT), then
                # broadcast den across partitions on the idle gpsimd so
                # neither PE nor the DVE queue head ever waits on it.
                # per-head order: head0's psum bank frees before head1's
                # copies start, unblocking the next tile's first Y matmul
                for h in range(2):
                    nc.vector.tensor_copy(den_t[h][qt][:], yt_ps[h][64:65, :])
                    nc.vector.tensor_copy(YTu_t[h][qt], yt_ps[h][0:64, :])
                den_bc = []
                for h in range(2):
                    bc = norm.tile(
                        [64, QT_TILE], F32, tag=f"denbc{h}", name=f"denbc{h}"
                    )
                    nc.gpsimd.partition_broadcast(bc, den_t[h][qt][:])
                    den_bc.append(bc)
                return den_bc

            def emit_norm_b(qt, den_bc):
                # ~51-ULP reciprocal (5x faster than the iterative divide),
                # then scale the unnormalized attention rows.
                for h in range(2):
                    rec_sb = norm.tile(
                        [64, QT_TILE], F32, tag=f"rec{h}", name=f"rec{h}"
                    )
                    nc.vector.reciprocal_approx_fast(rec_sb, den_bc[h])
                    nc.vector.tensor_mul(
                        YTn_t[qt][64 * h : 64 * h + 64, :],
                        YTu_t[h][qt][:],
                        rec_sb,
                    )

            def emit_norm_tail(qt, yt_ps):
                # last tile: PE is idle, so broadcast the denominator with a
                # K=1 matmul and normalize straight out of PSUM -- shortest
                # serial chain before the final out-projection.
                ones_r = norm.tile([1, 64], F32R, tag="onesr", name="onesr")
                with nc.allow_low_precision(reason="f32r ones for tail bcast"):
                    nc.vector.tensor_copy(ones_r, ones_f32[0:1, 0:64])
                for h in range(2):
                    den_r = norm.tile(
                        [1, QT_TILE], F32R, tag=f"denr{h}", name=f"denr{h}"
                    )
                    with nc.allow_low_precision(
                        reason="tf32-rounded softmax denominator for the "
                        "tail broadcast matmul"
                    ):
                        nc.vector.tensor_copy(den_r, yt_ps[h][64:65, :])
                    bc_ps = ps_proj.tile(
                        [64, QT_TILE], F32, tag="psproj", name="bc_tail"
                    )
                    nc.tensor.matmul(
                        bc_ps,
                        ones_r,
                        den_r,
                        start=True,
                        stop=True,
                    )
                    rec_sb = norm.tile(
                        [64, QT_TILE], F32, tag=f"rec{h}", name=f"rect{h}"
                    )
                    nc.vector.reciprocal_approx_fast(rec_sb, bc_ps)
                    nc.vector.tensor_mul(
                        YTn_t[qt][64 * h : 64 * h + 64, :],
                        yt_ps[h][0:64, :],
                        rec_sb,
                    )

            def emit_outproj_sv(qt, sv):
                tc8 = qt * (QT_TILE // 128) + sv
                ps_o = ps_proj.tile([128, C], F32, tag="psproj", name="ps_o")
                nc.tensor.matmul(
                    ps_o,
                    YTn_t[qt][:, bass.ts(sv, 128)],
                    woT_sb,
                    start=True,
                    stop=True,
                )
                o_sb = osb.tile([128, C], BF16, tag="osb")
                nc.vector.tensor_copy(o_sb, ps_o)
                nc.sync.dma_start(out[bass.ts(tc8, 128), :], o_sb)

            xt_tiles = {0: xt_first}

            def emit_xt(i):
                if i not in xt_tiles and i < N_QT:
                    xt_i = xin.tile(
                        [128, CK, QT_TILE], BF16, tag="xt", name=f"xt{i}"
                    )
                    nc.sync.dma_start(xt_i, xT_ap[:, :, bass.ts(i, QT_TILE)])
                    xt_tiles[i] = xt_i

            def emit_s_exp(qt2, pair):
                """S^T quad + exp + causal mask for (query tile qt2, chunk
                pair). The S psum tile is per-CHUNK (both heads): the two
                matmuls filling it hit different PE row groups (lhsT base
                partitions 0 / 64) and run concurrently in the array, and
                the scheduler keeps them adjacent because each exp is
                unblocked by one tile. Diagonal chunks skip the fully-
                masked 128r query prefix (excluded from Y, never read) and
                mask only the [128,128] triangle block."""
                e_list = []
                for sub in range(2):
                    c = pair * 2 + sub
                    r = c - 4 * qt2
                    off = KC * r if r > 0 else 0
                    s_ps = ps_s.tile(
                        [128, 2, QT_TILE], F32, tag="s", name=f"s{sub}"
                    )
                    for h in range(2):
                        hp = slice(h * 64, h * 64 + 64)
                        nc.tensor.matmul(
                            s_ps[:, h, off:],
                            KT_t[c // 4][hp, bass.ts(c % 4, KC)],
                            QT_t[qt2][hp, off:],
                            start=True,
                            stop=True,
                        )
                    e_sb = exps.tile(
                        [128, 2, QT_TILE], BF16, tag="e", name=f"e{sub}"
                    )
                    nc.scalar.activation(
                        e_sb[:, :, off:],
                        s_ps[:, :, off:],
                        mybir.ActivationFunctionType.Exp,
                        scale=SCALE,
                    )
                    if r >= 0:
                        for h in range(2):
                            nc.vector.tensor_mul(
                                e_sb[:, h, bass.ts(r, KC)],
                                e_sb[:, h, bass.ts(r, KC)],
                                tri_bf,
                            )
                    e_list.append(e_sb)
                return e_list

            qproj_done = set()
            den_bcs = {}
            pending_e = {}
            for i in range(1, N_QT):
                emit_xt(i)
            for qt in range(N_QT):
                xt = xt_tiles[qt]
                if qt not in qproj_done:
                    emit_qproj(qt, xt)
                    qproj_done.add(qt)
                if qt == 0:
                    emit_kproj(qt, xt)
                    pending_e[(0, 0)] = emit_s_exp(0, 0)
                    for sv in range(4):
                        emit_vproj(qt, xt, sv)

                yt_ps = [
                    ps_yt.tile([128, QT_TILE], F32, tag=f"yt{h}", name=f"yt{h}")
                    for h in range(2)
                ]
                n_pairs = 2 * (qt + 1)
                outproj_at = {}
                for sv in range(4):
                    outproj_at.setdefault(min(3 + sv, n_pairs - 1), []).append(sv)
                for pair in range(n_pairs):
                    e_sb = pending_e.pop((qt, pair), None)
                    if e_sb is None:
                        e_sb = emit_s_exp(qt, pair)
                    if pair == min(2, n_pairs - 1) and qt + 1 < N_QT:
                        emit_qproj(qt + 1, xt_tiles[qt + 1])
                        qproj_done.add(qt + 1)
                    # 1-deep S pipeline: emit the NEXT pair's S quad + exp
                    # before this pair's projections and Y matmuls, so ACT
                    # always has the next exp input ready.
                    if pair + 1 < n_pairs:
                        pending_e[(qt, pair + 1)] = emit_s_exp(qt, pair + 1)
                    elif qt + 1 < N_QT:
                        pending_e[(qt + 1, 0)] = emit_s_exp(qt + 1, 0)
                    # pipelined projections / out-proj for other tiles
                    if pair == 0 and qt > 0:
                        emit_kproj(qt, xt)
                    if qt > 0 and pair < 4:
                        emit_vproj(qt, xt, pair)
                    for h in range(2):
                        for sub in range(2):
                            c = pair * 2 + sub
                            r = c - 4 * qt
                            off = KC * r if r > 0 else 0
                            nc.tensor.matmul(
                                yt_ps[h][0:65, off:],
                                V_t[c // 4][:, c % 4, h * 65 : h * 65 + 65],
                                e_sb[sub][:, h, off:],
                                start=(pair == 0 and sub == 0),
                                stop=(pair == n_pairs - 1 and sub == 1),
                            )
                    if pair == 1 and qt > 0:
                        emit_norm_b(qt - 1, den_bcs[qt - 1])
                    if qt > 0:
                        for sv in outproj_at.get(pair, []):
                            emit_outproj_sv(qt - 1, sv)

                # ---- evict yt_ps fast + deferred normalization ----
                if qt < N_QT - 1:
                    den_bcs[qt] = emit_norm_a(qt, yt_ps)
                else:
                    emit_norm_tail(qt, yt_ps)
            for sv in range(4):
                emit_outproj_sv(N_QT - 1, sv)

    return nc


_PROGRAM = None


def _get_program():
    global _PROGRAM
    if _PROGRAM is None:
        _PROGRAM = build_program()
        if not _PROGRAM.is_finalized():
            _PROGRAM.finalize()
    return _PROGRAM


def make_in_maps(x, w_qkv, b_qkv, w_out, b_out):
    """Shard the full inputs into per-core input maps."""
    import ml_dtypes

    bf16 = ml_dtypes.bfloat16
    x = np.ascontiguousarray(x, dtype=np.float32)
    w_qkv = np.ascontiguousarray(w_qkv, dtype=np.float32)
    b_qkv = np.ascontiguousarray(b_qkv, dtype=np.float32)
    w_out = np.ascontiguousarray(w_out, dtype=np.float32)

    wq = w_qkv[0:C]  # [C, C] rows = q features
    wk = w_qkv[C : 2 * C]
    wv = w_qkv[2 * C : 3 * C]
    bq_full = b_qkv[0:C]
    bk_full = b_qkv[C : 2 * C]
    bv_full = b_qkv[2 * C : 3 * C]

    xT_b = [np.ascontiguousarray(x[b].T.astype(bf16)) for b in range(B)]

    in_maps = []
    for core in range(N_CORES):
        b = core // 4
        g = core % 4
        rows = slice(g * HD, (g + 1) * HD)  # this core's head dims
        woT = np.ascontiguousarray(w_out[:, rows].T.astype(bf16))  # [HD, C]
        in_maps.append(
            {
                "xT": xT_b[b],
                "wqT": np.ascontiguousarray(wq[rows].T.astype(bf16)),
                "wkT": np.ascontiguousarray(wk[rows].T.astype(bf16)),
                "wvT": np.ascontiguousarray(wv[rows].T.astype(bf16)),
                "woT": woT,
                "bq": np.ascontiguousarray(bq_full[rows]),
                "bk": np.ascontiguousarray(bk_full[rows]),
                "bv": np.ascontiguousarray(bv_full[rows]),
            }
        )
    return in_maps


def kernel(x, w_qkv, b_qkv, w_out, b_out, _trace=False, _trace_kwargs=None):
    in_maps = make_in_maps(x, w_qkv, b_qkv, w_out, b_out)
    nc = _get_program()
    res = run_bass_kernel_spmd(
        nc,
        in_maps,
        list(range(N_CORES)),
        trace=_trace,
        **(_trace_kwargs or {}),
    )
    outs = [res.results[c]["out"].astype(np.float32) for c in range(N_CORES)]
    bo = np.asarray(b_out, dtype=np.float32)
    # unshard: sum the 4 row-parallel partials per batch (+ bias), stack
    y = np.stack(
        [
            outs[0] + outs[1] + outs[2] + outs[3] + bo,
            outs[4] + outs[5] + outs[6] + outs[7] + bo,
        ]
    ).astype(np.float32)
    if _trace:
        return y, res
    return y


# revision 75
# speedup vs baseline: 1.0122x; 1.0109x over previous
"""Causal self-attention (B=2, T=4096, C=512, H=8, Dh=64) on 8 trn2 cores.

Sharding: core = (batch, head-pair). 2 batches x 4 head-pairs = 8 cores.
Each core computes q/k/v projections for its 2 heads, causal attention in
S^T ([k, q]) layout, and a row-parallel slice of the output projection.
Host sums the 4 bf16 partial outputs per batch (+ b_out) and stacks.

bf16 pipeline (PSUM accumulation stays f32 where it matters), measured
~183 us/core on trn2 (from a 364 us starting point; ACT exp floor ~139us,
PE ~153us busy at 93% density):
  - x / weights arrive bf16; Q/K/V produced bf16 (projection matmuls
    accumulate f32 in PSUM, DVE bias-add casts to bf16).
  - S^T chunk psum is per-CHUNK [128, 2(heads), 512]: the two matmuls
    filling it sit on different PE row groups (lhsT base partition 0 /
    64, K=64 each) and execute CONCURRENTLY in the array (~2x S
    throughput); the scheduler keeps them adjacent because each exp is
    unblocked by one tile.
  - exp on ACT per chunk at [128, <=1024] (PSUM source, scale fused);
    diagonal chunks narrow S/exp/Y to skip the fully-masked 128r query
    prefix, and only the [128,128] triangle block is mask-multiplied
    (DVE bf16).
  - YT[h][65, 512] += V_chunk @ expS in f32 PSUM (row 64 = softmax
    denominator via an appended ones column in V).
  - Deferred normalization: yt_ps is evicted immediately (bf16 YTu +
    den row), the denominator is partition-broadcast on the otherwise
    idle GPSIMD, and the ~51-ULP reciprocal_approx_fast (5x faster than
    the iterative DVE divide) + scale run one tile behind, so neither
    PE nor the DVE queue head ever waits on the reciprocal. The last
    tile instead uses a PE K=1 broadcast matmul + PSUM-direct ops for
    the shortest serial tail.
  - Out-projection: both heads stacked on partitions (YTn [128, 512],
    woT [128, 512]) -> ONE K=128 matmul per 128-query block.
  - 1-deep S software pipeline: each pair body first emits the NEXT
    pair's S quad + exp (crossing tile boundaries), so ACT always has
    its next input in flight and PE/ACT never idle against each other.
"""

import os
import sys

import numpy as np

for _p in ("/opt/trn_rl_repo",):
    if os.path.isdir(_p) and _p not in sys.path:
        sys.path.insert(0, _p)

os.environ.setdefault("MYCRO_LOCAL_CACHE", "1")


def _ensure_ntff_hook():
    """bass_utils' trace path imports antenv.axon_hooks; some images lack
    it. Recreate the module with the same ctypes hook if missing."""
    try:
        import antenv.axon_hooks  # noqa: F401

        return
    except ImportError:
        pass
    try:
        import types

        import antenv  # noqa: F401
        from trn_agent_boot.trn_boot import _ntff_profile_via_ctypes

        hook = _ntff_profile_via_ctypes("/opt/axon/libaxon_pjrt.so")
        mod = types.ModuleType("antenv.axon_hooks")
        mod.get_axon_ntff_profile_hook = lambda: hook
        mod.set_axon_ntff_profile_hook = lambda h: None
        sys.modules["antenv.axon_hooks"] = mod
    except Exception:
        pass


_ensure_ntff_hook()

import concourse.bass as bass  # noqa: E402
from concourse import bacc  # noqa: E402
import concourse.mybir as mybir  # noqa: E402
import concourse.tile as tile  # noqa: E402
from concourse.bass_utils import run_bass_kernel_spmd  # noqa: E402

F32 = mybir.dt.float32
F32R = mybir.dt.float32r
BF16 = mybir.dt.bfloat16

B, T, C, H, DH = 2, 4096, 512, 8, 64
HEADS_PER_CORE = 2
HD = HEADS_PER_CORE * DH  # 128: head dims owned by one core
N_CORES = 8
QT_TILE = 512  # queries per attention tile
KC = 128  # keys per chunk (contraction granularity)
N_QT = T // QT_TILE  # 8
N_KC = T // KC  # 32
CK = C // 128  # 4 contraction chunks for the projections
SCALE = 1.0 / float(np.sqrt(DH))


def build_program():
    nc = bacc.Bacc(None)

    xT = nc.declare_dram_parameter("xT", [C, T], BF16, isOutput=False)
    wqT = nc.declare_dram_parameter("wqT", [C, HD], BF16, isOutput=False)
    wkT = nc.declare_dram_parameter("wkT", [C, HD], BF16, isOutput=False)
    wvT = nc.declare_dram_parameter("wvT", [C, HD], BF16, isOutput=False)
    # woT[p, j]: rows of w_out for this core's head dims; rows 0-63 = head0
    # dims, 64-127 = head1 dims (matches the stacked YTn layout, so the
    # out-projection is ONE K=128 matmul summing both heads).
    woT = nc.declare_dram_parameter("woT", [HD, C], BF16, isOutput=False)
    bq = nc.declare_dram_parameter("bq", [HD], F32, isOutput=False)
    bk = nc.declare_dram_parameter("bk", [HD], F32, isOutput=False)
    bv = nc.declare_dram_parameter("bv", [HD], F32, isOutput=False)
    out = nc.declare_dram_parameter("out", [T, C], BF16, isOutput=True)

    with tile.TileContext(nc) as tc:
        with (
            tc.tile_pool(name="singles", bufs=1) as singles,
            tc.tile_pool(name="xin", bufs=8) as xin,
            tc.tile_pool(name="exps", bufs=8) as exps,
            tc.tile_pool(name="osb", bufs=5) as osb,
            tc.tile_pool(name="norm", bufs=3) as norm,
            tc.tile_pool(name="ps_proj", bufs=2, space="PSUM") as ps_proj,
            tc.tile_pool(name="ps_s", bufs=2, space="PSUM") as ps_s,
            tc.tile_pool(name="ps_yt", bufs=1, space="PSUM") as ps_yt,
        ):
            # ---- resident inputs (x0 + q/k weights first: they gate the
            # first PE work) --------------------------------------------
            xT_ap = xT.rearrange("(ko p) t -> p ko t", p=128)
            xt_first = xin.tile([128, CK, QT_TILE], BF16, tag="xt", name="xt_first")
            nc.sync.dma_start(xt_first, xT_ap[:, :, bass.ts(0, QT_TILE)])
            wqT_sb = singles.tile([128, CK, HD], BF16)
            nc.sync.dma_start(wqT_sb, wqT.rearrange("(ko p) m -> p ko m", p=128))
            wkT_sb = singles.tile([128, CK, HD], BF16)
            nc.sync.dma_start(wkT_sb, wkT.rearrange("(ko p) m -> p ko m", p=128))
            wvT_sb = singles.tile([128, CK, HD], BF16)
            nc.sync.dma_start(wvT_sb, wvT.rearrange("(ko p) m -> p ko m", p=128))
            woT_sb = singles.tile([HD, C], BF16)
            nc.sync.dma_start(woT_sb, woT[:])

            bq_col = singles.tile([128, 1], F32)
            nc.sync.dma_start(bq_col, bq.rearrange("(p one) -> p one", one=1))
            bk_col = singles.tile([128, 1], F32)
            nc.sync.dma_start(bk_col, bk.rearrange("(p one) -> p one", one=1))
            bv_row = singles.tile([1, HD], F32)
            nc.sync.dma_start(bv_row, bv[None, :])

            ones_f32 = singles.tile([128, 128], F32)
            nc.vector.memset(ones_f32, 1.0)


            ones_bf = singles.tile([128, 4], BF16)
            nc.vector.tensor_copy(ones_bf, ones_f32[:, 0:4])

            # [128,128] causal triangle: tri[k, qq] = 1 if k <= qq.
            # Built in F32 (affine_select needs it), then cast.
            tri_f32 = singles.tile([128, 128], F32)
            nc.vector.memset(tri_f32, 1.0)
            nc.gpsimd.affine_select(
                out=tri_f32,
                in_=tri_f32,
                compare_op=mybir.AluOpType.is_ge,
                fill=0.0,
                base=0,
                pattern=[[1, 128]],
                channel_multiplier=-1,
            )
            tri_bf = singles.tile([128, 128], BF16)
            nc.vector.tensor_copy(tri_bf, tri_f32)

            # broadcast bv across partitions on gpsimd
            bias_v_sb = singles.tile([128, HD], F32)
            nc.gpsimd.partition_broadcast(bias_v_sb, bv_row)
            bias_v2 = bias_v_sb.rearrange("p (h x) -> p h x", h=2)

            # per-tile storage (separate tile objects -> precise deps)
            QT_t = [
                singles.tile([128, QT_TILE], BF16, name=f"qtt{i}", tag=f"qtt{i}")
                for i in range(N_QT)
            ]
            KT_t = [
                singles.tile([128, QT_TILE], BF16, name=f"ktt{i}", tag=f"ktt{i}")
                for i in range(N_QT)
            ]
            # V chunks in [k, d] layout; per tile: 4 chunks of
            # [V0 | ones | V1 | ones] (65-column stride per head slice)
            V_t = [
                singles.tile([128, 4, 130], BF16, name=f"vt{i}", tag=f"vt{i}")
                for i in range(N_QT)
            ]
            # unnormalized attention outputs + denominators (deferred norm)
            YTu_t = [
                [
                    singles.tile(
                        [64, QT_TILE], BF16, name=f"ytu{h}_{i}", tag=f"ytu{h}_{i}"
                    )
                    for i in range(N_QT)
                ]
                for h in range(2)
            ]
            den_t = [
                [
                    singles.tile([1, QT_TILE], F32, name=f"den{h}_{i}", tag=f"den{h}_{i}")
                    for i in range(N_QT)
                ]
                for h in range(2)
            ]
            # normalized YT, both heads stacked on partitions (h0: 0-63,
            # h1: 64-127) so the out-projection contracts K=128 in one shot
            YTn_t = [
                singles.tile([128, QT_TILE], BF16, name=f"ytn{i}", tag=f"ytn{i}")
                for i in range(N_QT)
            ]
            for i in range(N_QT):
                nc.vector.tensor_copy(V_t[i][:, :, 64:65], ones_bf[:, :, None])
                nc.vector.tensor_copy(V_t[i][:, :, 129:130], ones_bf[:, :, None])

            def emit_qproj(qt, xt):
                ps_q = ps_proj.tile([128, QT_TILE], F32, tag="psproj", name="ps_q")
                for kc in range(CK):
                    nc.tensor.matmul(
                        ps_q,
                        wqT_sb[:, kc, :],
                        xt[:, kc, :],
                        start=(kc == 0),
                        stop=(kc == CK - 1),
                    )
                nc.vector.tensor_scalar_add(QT_t[qt][:], ps_q, bq_col)

            def emit_kproj(qt, xt):
                ps_k = ps_proj.tile([128, QT_TILE], F32, tag="psproj", name="ps_k")
                for kc in range(CK):
                    nc.tensor.matmul(
                        ps_k,
                        wkT_sb[:, kc, :],
                        xt[:, kc, :],
                        start=(kc == 0),
                        stop=(kc == CK - 1),
                    )
                nc.vector.tensor_scalar_add(KT_t[qt][:], ps_k, bk_col)

            def emit_vproj(qt, xt, sv):
                ps_v = ps_proj.tile([128, HD], F32, tag="psproj", name="ps_v")
                for kc in range(CK):
                    nc.tensor.matmul(
                        ps_v,
                        xt[:, kc, bass.ts(sv, 128)],
                        wvT_sb[:, kc, :],
                        start=(kc == 0),
                        stop=(kc == CK - 1),
                    )
                vt = V_t[qt]
                v_vals = bass.AP(
                    tensor=vt.tensor,
                    offset=vt.offset,
                    ap=[vt.ap[0], vt.ap[1], [65, 2], [1, 64]],
                )
                nc.vector.tensor_add(
                    v_vals[:, sv],
                    ps_v.rearrange("p (h x) -> p h x", h=2),
                    bias_v2,
                )

            def emit_norm_a(qt, yt_ps):
                # evict yt_ps fast (den row + unnormalized YT), then
                # broadcast den across partitions on the idle gpsimd so
                # neither PE nor the DVE queue head ever waits on it.
                # per-head order: head0's psum bank frees before head1's
                # copies start, unblocking the next tile's first Y matmul
                for h in range(2):
                    nc.vector.tensor_copy(den_t[h][qt][:], yt_ps[h][64:65, :])
                    nc.vector.tensor_copy(YTu_t[h][qt], yt_ps[h][0:64, :])
                den_bc = []
                for h in range(2):
                    bc = norm.tile(
                        [64, QT_TILE], F32, tag=f"denbc{h}", name=f"denbc{h}"
                    )
                    nc.gpsimd.partition_broadcast(bc, den_t[h][qt][:])
                    den_bc.append(bc)
                return den_bc

            def emit_norm_b(qt, den_bc):
                # ~51-ULP reciprocal (5x faster than the iterative divide),
                # then scale the unnormalized attention rows.
                for h in range(2):
                    rec_sb = norm.tile(
                        [64, QT_TILE], F32, tag=f"rec{h}", name=f"rec{h}"
                    )
                    nc.vector.reciprocal_approx_fast(rec_sb, den_bc[h])
                    nc.vector.tensor_mul(
                        YTn_t[qt][64 * h : 64 * h + 64, :],
                        YTu_t[h][qt][:],
                        rec_sb,
                    )

            def emit_norm_tail(qt, yt_ps):
                # last tile: PE is idle, so broadcast the denominator with a
                # K=1 matmul and normalize straight out of PSUM -- shortest
                # serial chain before the final out-projection.
                ones_r = norm.tile([1, 64], F32R, tag="onesr", name="onesr")
                with nc.allow_low_precision(reason="f32r ones for tail bcast"):
                    nc.vector.tensor_copy(ones_r, ones_f32[0:1, 0:64])
                for h in range(2):
                    den_r = norm.tile(
                        [1, QT_TILE], F32R, tag=f"denr{h}", name=f"denr{h}"
                    )
                    with nc.allow_low_precision(
                        reason="tf32-rounded softmax denominator for the "
                        "tail broadcast matmul"
                    ):
                        nc.vector.tensor_copy(den_r, yt_ps[h][64:65, :])
                    bc_ps = ps_proj.tile(
                        [64, QT_TILE], F32, tag="psproj", name="bc_tail"
                    )
                    nc.tensor.matmul(
                        bc_ps,
                        ones_r,
                        den_r,
                        start=True,
                        stop=True,
                    )
                    rec_sb = norm.tile(
                        [64, QT_TILE], F32, tag=f"rec{h}", name=f"rect{h}"
                    )
                    nc.vector.reciprocal_approx_fast(rec_sb, bc_ps)
                    nc.vector.tensor_mul(
                        YTn_t[qt][64 * h : 64 * h + 64, :],
                        yt_ps[h][0:64, :],
                        rec_sb,
                    )

            def emit_outproj_sv(qt, sv):
                tc8 = qt * (QT_TILE // 128) + sv
                ps_o = ps_proj.tile([128, C], F32, tag="psproj", name="ps_o")
                nc.tensor.matmul(
                    ps_o,
                    YTn_t[qt][:, bass.ts(sv, 128)],
                    woT_sb,
                    start=True,
                    stop=True,
                )
                o_sb = osb.tile([128, C], BF16, tag="osb")
                nc.vector.tensor_copy(o_sb, ps_o)
                nc.sync.dma_start(out[bass.ts(tc8, 128), :], o_sb)

            xt_tiles = {0: xt_first}

            def emit_xt(i):
                if i not in xt_tiles and i < N_QT:
                    xt_i = xin.tile(
                        [128, CK, QT_TILE], BF16, tag="xt", name=f"xt{i}"
                    )
                    nc.sync.dma_start(xt_i, xT_ap[:, :, bass.ts(i, QT_TILE)])
                    xt_tiles[i] = xt_i

            def emit_s_exp(qt2, pair):
                """S^T quad + exp + causal mask for (query tile qt2, chunk
                pair). The S psum tile is per-CHUNK (both heads): the two
                matmuls filling it hit different PE row groups (lhsT base
                partitions 0 / 64) and run concurrently in the array, and
                the scheduler keeps them adjacent because each exp is
                unblocked by one tile. Diagonal chunks skip the fully-
                masked 128r query prefix (excluded from Y, never read) and
                mask only the [128,128] triangle block."""
                e_list = []
                for sub in range(2):
                    c = pair * 2 + sub
                    r = c - 4 * qt2
                    off = KC * r if r > 0 else 0
                    s_ps = ps_s.tile(
                        [128, 2, QT_TILE], F32, tag="s", name=f"s{sub}"
                    )
                    for h in range(2):
                        hp = slice(h * 64, h * 64 + 64)
                        nc.tensor.matmul(
                            s_ps[:, h, off:],
                            KT_t[c // 4][hp, bass.ts(c % 4, KC)],
                            QT_t[qt2][hp, off:],
                            start=True,
                            stop=True,
                        )
                    e_sb = exps.tile(
                        [128, 2, QT_TILE], BF16, tag="e", name=f"e{sub}"
                    )
                    nc.scalar.activation(
                        e_sb[:, :, off:],
                        s_ps[:, :, off:],
                        mybir.ActivationFunctionType.Exp,
                        scale=SCALE,
                    )
                    if r >= 0:
                        for h in range(2):
                            nc.vector.tensor_mul(
                                e_sb[:, h, bass.ts(r, KC)],
                                e_sb[:, h, bass.ts(r, KC)],
                                tri_bf,
                            )
                    e_list.append(e_sb)
                return e_list

            qproj_done = set()
            den_bcs = {}
            pending_e = {}
            for i in range(1, N_QT):
                emit_xt(i)
            for qt in range(N_QT):
                xt = xt_tiles[qt]
                if qt not in qproj_done:
                    emit_qproj(qt, xt)
                    qproj_done.add(qt)
                if qt == 0:
                    emit_kproj(qt, xt)
                    pending_e[(0, 0)] = emit_s_exp(0, 0)
                    for sv in range(4):
                        emit_vproj(qt, xt, sv)

                yt_ps = [
                    ps_yt.tile([128, QT_TILE], F32, tag=f"yt{h}", name=f"yt{h}")
                    for h in range(2)
                ]
                n_pairs = 2 * (qt + 1)
                outproj_at = {}
                for sv in range(4):
                    outproj_at.setdefault(min(3 + sv, n_pairs - 1), []).append(sv)
                for pair in range(n_pairs):
                    e_sb = pending_e.pop((qt, pair), None)
                    if e_sb is None:
                        e_sb = emit_s_exp(qt, pair)
                    if pair == min(2, n_pairs - 1) and qt + 1 < N_QT:
                        emit_qproj(qt + 1, xt_tiles[qt + 1])
                        qproj_done.add(qt + 1)
                    # 1-deep S pipeline: emit the NEXT pair's S quad + exp
                    # before this pair's projections and Y matmuls, so ACT
                    # always has the next exp input ready.
                    if pair + 1 < n_pairs:
                        pending_e[(qt, pair + 1)] = emit_s_exp(qt, pair + 1)
                    elif qt + 1 < N_QT:
                        pending_e[(qt + 1, 0)] = emit_s_exp(qt + 1, 0)
                    # pipelined projections / out-proj for other tiles
                    if pair == 0 and qt > 0:
                        emit_kproj(qt, xt)
                    if qt > 0 and pair < 4:
                        emit_vproj(qt, xt, pair)
                    for h in range(2):
                        for sub in range(2):
                            c = pair * 2 + sub
                            r = c - 4 * qt
                            off = KC * r if r > 0 else 0
                            nc.tensor.matmul(
                                yt_ps[h][0:65, off:],
                                V_t[c // 4][:, c % 4, h * 65 : h * 65 + 65],
                                e_sb[sub][:, h, off:],
                                start=(pair == 0 and sub == 0),
                                stop=(pair == n_pairs - 1 and sub == 1),
                            )
                    if pair == 1 and qt > 0:
                        emit_norm_b(qt - 1, den_bcs[qt - 1])
                    if qt > 0:
                        for sv in outproj_at.get(pair, []):
                            emit_outproj_sv(qt - 1, sv)

                # ---- evict yt_ps fast + deferred normalization ----
                if qt < N_QT - 1:
                    den_bcs[qt] = emit_norm_a(qt, yt_ps)
                else:
                    emit_norm_tail(qt, yt_ps)
            for sv in range(4):
                emit_outproj_sv(N_QT - 1, sv)

    return nc


_PROGRAM = None


def _get_program():
    global _PROGRAM
    if _PROGRAM is None:
        _PROGRAM = build_program()
        if not _PROGRAM.is_finalized():
            _PROGRAM.finalize()
    return _PROGRAM


def make_in_maps(x, w_qkv, b_qkv, w_out, b_out):
    """Shard the full inputs into per-core input maps."""
    import ml_dtypes

    bf16 = ml_dtypes.bfloat16
    x = np.ascontiguousarray(x, dtype=np.float32)
    w_qkv = np.ascontiguousarray(w_qkv, dtype=np.float32)
    b_qkv = np.ascontiguousarray(b_qkv, dtype=np.float32)
    w_out = np.ascontiguousarray(w_out, dtype=np.float32)

    wq = w_qkv[0:C]  # [C, C] rows = q features
    wk = w_qkv[C : 2 * C]
    wv = w_qkv[2 * C : 3 * C]
    bq_full = b_qkv[0:C]
    bk_full = b_qkv[C : 2 * C]
    bv_full = b_qkv[2 * C : 3 * C]

    xT_b = [np.ascontiguousarray(x[b].T.astype(bf16)) for b in range(B)]

    in_maps = []
    for core in range(N_CORES):
        b = core // 4
        g = core % 4
        rows = slice(g * HD, (g + 1) * HD)  # this core's head dims
        woT = np.ascontiguousarray(w_out[:, rows].T.astype(bf16))  # [HD, C]
        in_maps.append(
            {
                "xT": xT_b[b],
                "wqT": np.ascontiguousarray(wq[rows].T.astype(bf16)),
                "wkT": np.ascontiguousarray(wk[rows].T.astype(bf16)),
                "wvT": np.ascontiguousarray(wv[rows].T.astype(bf16)),
                "woT": woT,
                "bq": np.ascontiguousarray(bq_full[rows]),
                "bk": np.ascontiguousarray(bk_full[rows]),
                "bv": np.ascontiguousarray(bv_full[rows]),
            }
        )
    return in_maps


def kernel(x, w_qkv, b_qkv, w_out, b_out, _trace=False, _trace_kwargs=None):
    in_maps = make_in_maps(x, w_qkv, b_qkv, w_out, b_out)
    nc = _get_program()
    res = run_bass_kernel_spmd(
        nc,
        in_maps,
        list(range(N_CORES)),
        trace=_trace,
        **(_trace_kwargs or {}),
    )
    outs = [res.results[c]["out"].astype(np.float32) for c in range(N_CORES)]
    bo = np.asarray(b_out, dtype=np.float32)
    # unshard: sum the 4 row-parallel partials per batch (+ bias), stack
    y = np.stack(
        [
            outs[0] + outs[1] + outs[2] + outs[3] + bo,
            outs[4] + outs[5] + outs[6] + outs[7] + bo,
        ]
    ).astype(np.float32)
    if _trace:
        return y, res
    return y
